# revision 33
# baseline (speedup 1.0000x reference)
"""Trainium2 Bass kernel for a 2-layer Mamba LM (B=2, L=1024, D=512,
d_inner=1024, d_state=16, vocab=32000) on 8 NeuronCores.

Sharding: d_inner tensor-parallel for the Mamba blocks (each core owns 128
of the 1024 inner channels; AllReduce for x_proj (64x2048) and out_proj
(512x2048) partial sums), vocab-sharded LM head (4000 rows per core, no
collective), embedding gathered per-core via indirect DMA.

v2: the whole network is software-pipelined over 4 token chunks of 512.
Each layer processes chunk k end-to-end (rms -> in_proj -> conv -> x_proj
-> AllReduce -> scan (state carried across chunks) -> gate -> out_proj ->
AllReduce -> residual), so the per-chunk AllReduces overlap compute of the
neighboring chunks.  All big matmuls run fp32r (1 cyc/row).  The LM head
runs in bf16 with fully preloaded weights and consumes the final-layernorm
output chunk by chunk, overlapping the tail of layer 1.
"""
import numpy as np
from ml_dtypes import bfloat16

import concourse.bass as bass
import concourse.bacc as bacc
import concourse.mybir as mybir
import concourse.tile as tile
from concourse.masks import make_identity

# model dims
B, L = 2, 1024
DIM = 512
D_STATE = 16
D_INNER = 1024
DT_RANK = 32
VOCAB = 32000
N_LAYERS = 2
EPS = 1e-5

N_CORES = 8
CH = D_INNER // N_CORES          # 128 local channels
VSH = VOCAB // N_CORES           # 4000 local vocab rows
T = B * L                        # 2048 token columns (batch-major)
P = 128
CW = 512                         # token chunk width
NT = T // CW                     # 4 token chunks
ND = DIM // P                    # 4 dim tiles
NRT = CH // 8                    # 16 row-tiles (8 ch x 16 states each)
VC = 500                         # head vocab chunk
NVC = VSH // VC                  # 8 vocab chunks
F32 = mybir.dt.float32
F32R = mybir.dt.float32r
BF16 = mybir.dt.bfloat16
I32 = mybir.dt.int32
AF = mybir.ActivationFunctionType
OP = mybir.AluOpType
RG = [list(range(N_CORES))]


def _mm(nc, out, lhsT, rhs, **kw):
    if lhsT.dtype == F32:
        lhsT = lhsT.bitcast(F32R)
    if rhs.dtype == F32:
        rhs = rhs.bitcast(F32R)
    nc.tensor.matmul(out=out, lhsT=lhsT, rhs=rhs, **kw)


def build_program():
    nc = bacc.Bacc("TRN2", num_devices=N_CORES)
    # register EPS as a const AP so activation(bias=EPS) works
    _ct = nc.alloc_sbuf_tensor(f"const-float32-{EPS}", [128, 1], F32)
    nc.gpsimd.memset(_ct.ap(), EPS)
    nc.const_aps.aps[(F32, EPS)] = _ct.ap()
    nc.all_engine_barrier()

    # ---- DRAM I/O ----
    tok = nc.dram_tensor("tok", [T, 1], I32, kind="ExternalInput").ap()
    embed = nc.dram_tensor("embed", [VOCAB, DIM], F32, kind="ExternalInput").ap()
    lw = []  # per-layer weights
    for l in range(N_LAYERS):
        lw.append({
            "inwx": nc.dram_tensor(f"inwx{l}", [DIM, CH], F32R, kind="ExternalInput").ap(),
            "inwz": nc.dram_tensor(f"inwz{l}", [DIM, CH], F32R, kind="ExternalInput").ap(),
            "convw": nc.dram_tensor(f"convw{l}", [CH, 4], F32, kind="ExternalInput").ap(),
            "convb": nc.dram_tensor(f"convb{l}", [CH, 1], F32, kind="ExternalInput").ap(),
            "xpw": nc.dram_tensor(f"xpw{l}", [CH, 64], F32R, kind="ExternalInput").ap(),
            "dtw": nc.dram_tensor(f"dtw{l}", [DT_RANK, CH], F32R, kind="ExternalInput").ap(),
            "dtb": nc.dram_tensor(f"dtb{l}", [CH, 1], F32, kind="ExternalInput").ap(),
            "acols": nc.dram_tensor(f"acols{l}", [P, NRT], F32, kind="ExternalInput").ap(),
            "dp": nc.dram_tensor(f"dp{l}", [CH, 1], F32, kind="ExternalInput").ap(),
            "outw": nc.dram_tensor(f"outw{l}", [CH, DIM], F32R, kind="ExternalInput").ap(),
        })
    rrepd = nc.dram_tensor("rrep", [P, NRT * P], F32R, kind="ExternalInput").ap()
    rbcd = nc.dram_tensor("rbc", [64, 2 * P], F32R, kind="ExternalInput").ap()
    srepd = nc.dram_tensor("srep", [P, NRT * P], F32R, kind="ExternalInput").ap()
    onesrd = nc.dram_tensor("ones_row", [P, P], F32R, kind="ExternalInput").ap()
    onescd = nc.dram_tensor("ones_col", [P, 1], F32R, kind="ExternalInput").ap()
    headw = nc.dram_tensor("headw", [DIM, VSH], BF16, kind="ExternalInput").ap()
    logits = nc.dram_tensor("logits", [T, VSH], F32, kind="ExternalOutput").ap()

    with tile.TileContext(nc) as tc:
        with (
            nc.allow_low_precision(reason="fp32r/bf16 rounding intentional; "
                                   "rel-err budget is 2e-2"),
            tc.tile_pool(name="sb1", bufs=1) as sb1,
            tc.tile_pool(name="sb2", bufs=2) as sb2,
            tc.tile_pool(name="sbc", bufs=2) as sbc,
            tc.tile_pool(name="sbs", bufs=2) as sbs,
            tc.tile_pool(name="ps", bufs=3, space="PSUM") as ps,
            tc.tile_pool(name="dram", bufs=2, space="DRAM") as drp,
        ):
            # ---- warmup AllReduce (absorbs CC ring setup latency) ----
            wz = sb1.tile([64, 16], F32)
            nc.vector.memset(wz[:], 0.0)
            warm_l = drp.tile([64, 16], F32, tag="warm_l")
            warm_r = drp.tile([64, 16], F32, tag="warm_r", addr_space="Shared")
            nc.sync.dma_start(warm_l[:], wz[:])
            nc.gpsimd.collective_compute(
                "AllReduce", OP.add, replica_groups=RG,
                ins=[warm_l.opt()], outs=[warm_r.opt()])

            # ---- consts needed by the gather/transpose path ----
            ident = sb1.tile([P, P], F32)
            make_identity(nc, ident[:])
            rrep = sb1.tile([P, NRT * P], F32R)
            nc.sync.dma_start(rrep[:], rrepd[:])
            rbc = sb1.tile([64, 2 * P], F32R)
            nc.sync.dma_start(rbc[:], rbcd[:])
            srep = sb1.tile([P, NRT * P], F32R)
            nc.sync.dma_start(srep[:], srepd[:])
            ones128 = sb1.tile([P, P], F32R)
            nc.sync.dma_start(ones128[:], onesrd[:])
            ones_row = ones128[0:1, :]
            ones_col = sb1.tile([P, 1], F32R)
            nc.sync.dma_start(ones_col[:], onescd[:])

            # ---- embedding gather + transpose into hT (emitted first: it
            # gates the whole layer pipeline) ----
            hT = [sb1.tile([P, T], F32, tag=f"hT{d}", name=f"hT{d}") for d in range(ND)]
            for tt in range(T // P):
                idx_t = sb2.tile([P, 1], I32, tag="idx")
                nc.sync.dma_start(idx_t[:], tok[tt * P:(tt + 1) * P, :])
                g_t = sb2.tile([P, DIM], F32, tag="w512")
                nc.gpsimd.indirect_dma_start(
                    out=g_t[:], out_offset=None, in_=embed[:],
                    in_offset=bass.IndirectOffsetOnAxis(ap=idx_t[:, :1], axis=0),
                )
                for d in range(ND):
                    gT_ps = ps.tile([P, P], F32, tag="pf", space="PSUM", bufs=2)
                    nc.tensor.transpose(out=gT_ps[:], in_=g_t[:, d * P:(d + 1) * P],
                                        identity=ident[:])
                    dst = hT[d][:, tt * P:(tt + 1) * P].bitcast(F32R)
                    if (tt + d) % 2 == 0:
                        nc.vector.tensor_copy(dst, gT_ps[:])
                    else:
                        nc.scalar.copy(dst, gT_ps[:])

            # ---- layer weights (sync queue), head weights (scalar queue) ----
            w = []
            for l in range(N_LAYERS):
                d = {}
                for k, ap in lw[l].items():
                    if k in ("inwx", "inwz"):
                        tl_ = []
                        for kk in range(ND):
                            t_ = sb1.tile([P, CH], F32R, tag=f"{k}{l}_{kk}")
                            nc.sync.dma_start(t_[:], ap[kk * P:(kk + 1) * P, :])
                            tl_.append(t_)
                        d[k] = tl_
                    else:
                        t_ = sb1.tile(list(ap.shape), ap.dtype, tag=f"{k}{l}")
                        nc.sync.dma_start(t_[:], ap[:])
                        d[k] = t_
                # negated conv bias for the sigmoid-via-exp path
                cbn = sb1.tile([CH, 1], F32, tag=f"convbn{l}", name="cbn")
                nc.vector.tensor_scalar_mul(cbn[:], d["convb"][:], -1.0)
                d["convbn"] = cbn
                w.append(d)
            # head weights, fully preloaded (bf16, 4x[128,4000])
            hw = []
            for d in range(ND):
                hw_t = sb1.tile([P, VSH], BF16, tag=f"hw{d}", name=f"hw{d}")
                nc.scalar.dma_start(hw_t[:], headw[d * P:(d + 1) * P, :])
                hw.append(hw_t)

            # conv inputs, per batch (3-col causal halo at the front)
            xc = [sb1.tile([P, L + 3], F32, tag=f"xc{b_}", name=f"xc{b_}")
                  for b_ in range(B)]
            # scan carry columns (one per row-tile), written at even chunks
            carry = sbc.tile([P, NRT], F32, tag="carry", name="carry")

            # ---- layers: two-stage software pipeline over 4 chunks ----
            # front(l,k): stats -> in_proj -> conv -> x_proj -> AR_dbc
            # back(l,k):  delta -> scan -> gate -> out_proj -> AR_op
            # Emission order front(l,k+1) before back(l,k) puts AR_dbc(k+1)
            # ahead of AR_op(k) on the CC stream so the scan of k+1 never
            # waits on the out_proj reduce of k.  Residual adds are deferred
            # one slot so their AR wait never blocks an engine queue.

            def front(l, k):
                wl = w[l]
                sl = slice(k * CW, (k + 1) * CW)
                b_, o = divmod(k * CW, L)
                ssp = ps.tile([1, CW], F32, tag="pf", space="PSUM", bufs=2,
                              name="ssp")
                for d in range(ND):
                    hsq = sb2.tile([P, CW], F32, tag="hsq")
                    nc.scalar.activation(hsq[:].bitcast(F32R), hT[d][:, sl],
                                         AF.Square)
                    _mm(nc, out=ssp[:], lhsT=ones_col[:], rhs=hsq[:],
                        start=(d == 0), stop=(d == ND - 1))
                # inv = (mean+eps)^-1/2 = exp(-0.5*ln(.)) - exp/ln table only
                lnm_c = sbc.tile([1, CW], F32, tag="lnm", bufs=1)
                nc.scalar.activation(lnm_c[:], ssp[:],
                                     AF.Ln, bias=EPS, scale=1.0 / DIM)
                inv_c = sbc.tile([1, CW], F32, tag="inv")
                nc.scalar.activation(inv_c[:].bitcast(F32R), lnm_c[:],
                                     AF.Exp, scale=-0.5)
                bp = ps.tile([P, CW], F32, tag="pf", space="PSUM", bufs=2,
                             name="bp")
                _mm(nc, out=bp[:], lhsT=ones_row[:], rhs=inv_c[:],
                    start=True, stop=True)
                bcinv = sbc.tile([P, CW], F32, tag="bcinv")
                nc.scalar.copy(bcinv[:], bp[:])

                # in_proj (rms gamma folded into W; scale by inv on evac)
                if o == 0:
                    nc.vector.memset(xc[b_][:, 0:3], 0.0)
                px = ps.tile([P, CW], F32, tag="pf", space="PSUM", bufs=2,
                             name="px")
                pz = ps.tile([P, CW], F32, tag="pf", space="PSUM", bufs=2,
                             name="pz")
                for d in range(ND):
                    _mm(nc, out=px[:], lhsT=wl["inwx"][d][:],
                        rhs=hT[d][:, sl], start=(d == 0), stop=(d == ND - 1))
                for d in range(ND):
                    _mm(nc, out=pz[:], lhsT=wl["inwz"][d][:],
                        rhs=hT[d][:, sl], start=(d == 0), stop=(d == ND - 1))
                nc.vector.tensor_tensor(out=xc[b_][:, 3 + o:3 + o + CW],
                                        in0=px[:], in1=bcinv[:], op=OP.mult)
                z_c = sbc.tile([P, CW], F32, tag="z")
                nc.vector.tensor_tensor(out=z_c[:], in0=pz[:],
                                        in1=bcinv[:], op=OP.mult)
                # silu(z) denominator: 1/(1+exp(-z)) via exp table + DVE
                ez = sbc.tile([P, CW], F32, tag="ez", bufs=1)
                nc.scalar.activation(ez[:], z_c[:], AF.Exp, scale=-1.0)
                nc.scalar.activation(ez[:], ez[:], AF.Copy, bias=1.0)
                rz_c = sbc.tile([P, CW], F32, tag="rz")
                nc.vector.reciprocal(rz_c[:], ez[:])

                # causal depthwise conv + silu -> xs_c
                cv = sbc.tile([P, CW], F32, tag="cv", bufs=1, name="cv")
                nc.vector.tensor_scalar_mul(cv[:], xc[b_][:, o:o + CW],
                                            wl["convw"][:, 0:1])
                for kk in (1, 2, 3):
                    nc.vector.scalar_tensor_tensor(
                        out=cv[:], in0=xc[b_][:, o + kk:o + kk + CW],
                        scalar=wl["convw"][:, kk:kk + 1], in1=cv[:],
                        op0=OP.mult, op1=OP.add)
                ec = sb2.tile([P, CW], F32, tag="w512", name="ec")
                nc.scalar.activation(ec[:], cv[:], AF.Exp, scale=-1.0,
                                     bias=wl["convbn"][:, :1])
                nc.scalar.activation(ec[:], ec[:], AF.Copy, bias=1.0)
                rc = sb2.tile([P, CW], F32, tag="w512", name="rc")
                nc.vector.reciprocal(rc[:], ec[:])
                xs_c = sbc.tile([P, CW], F32, tag="xs")
                nc.vector.scalar_tensor_tensor(
                    out=xs_c[:].bitcast(F32R), in0=cv[:],
                    scalar=wl["convb"][:, :1], in1=rc[:],
                    op0=OP.add, op1=OP.mult)

                # x_proj partial + chunked AllReduce (trigger via scalar DMA)
                pd_ = ps.tile([64, CW], F32, tag="pf", space="PSUM", bufs=2,
                              name="pd_")
                _mm(nc, out=pd_[:], lhsT=wl["xpw"][:], rhs=xs_c[:],
                    start=True, stop=True)
                dbev = sb2.tile([64, CW], F32, tag="w512", name="dbev")
                nc.scalar.copy(dbev[:], pd_[:])
                dbc_l = drp.tile([64, CW], F32, tag="dbc_l", bufs=2)
                nc.scalar.dma_start(dbc_l[:], dbev[:])
                dbc_r = drp.tile([64, CW], F32, tag="dbc_r", bufs=2,
                                 addr_space="Shared")
                nc.gpsimd.collective_compute(
                    "AllReduce", OP.add, replica_groups=RG,
                    ins=[dbc_l.opt()], outs=[dbc_r.opt()])
                return dict(sl=sl, b_=b_, o=o, xs=xs_c, z=z_c, rz=rz_c,
                            dbc_r=dbc_r)

            def back(l, k, st, _unused):
                wl = w[l]
                sl, o = st["sl"], st["o"]
                xs_c, z_c, rz_c = st["xs"], st["z"], st["rz"]
                dbc = sbc.tile([64, CW], F32R, tag="dbc")
                nc.sync.dma_start(dbc[:], st["dbc_r"][:].bitcast(F32R))

                # delta = softplus(dt_w @ dbc[:32] + dt_b)
                pt = ps.tile([P, CW], F32, tag="pm", space="PSUM", bufs=3,
                             name="pt")
                _mm(nc, out=pt[:], lhsT=wl["dtw"][:], rhs=dbc[0:32, :],
                    start=True, stop=True)
                et = sb2.tile([P, CW], F32, tag="w512", name="et")
                nc.scalar.activation(et[:], pt[:], AF.Exp,
                                     bias=wl["dtb"][:, :1])
                delta = sbc.tile([P, CW], F32, tag="delta")
                nc.scalar.activation(delta[:].bitcast(F32R), et[:], AF.Ln,
                                     bias=1.0)
                dx = sbc.tile([P, CW], F32, tag="dx")
                nc.gpsimd.tensor_tensor(out=dx[:].bitcast(F32R), in0=delta[:],
                                        in1=xs_c[:], op=OP.mult)

                # B/C broadcast into (c,s)-row layout
                pb = ps.tile([P, CW], F32, tag="pm", space="PSUM", bufs=3,
                             name="pb")
                _mm(nc, out=pb[:], lhsT=rbc[32:64, 0:P], rhs=dbc[32:64, :],
                    start=True, stop=True)
                brep = sbc.tile([P, CW], F32, tag="brep")
                nc.scalar.copy(brep[:], pb[:])
                pc = ps.tile([P, CW], F32, tag="pm", space="PSUM", bufs=3,
                             name="pc")
                _mm(nc, out=pc[:], lhsT=rbc[32:64, P:2 * P], rhs=dbc[32:64, :],
                    start=True, stop=True)
                crep = sbc.tile([P, CW], F32, tag="crep")
                nc.scalar.copy(crep[:], pc[:])

                # row-tiles: replicate delta/dx, exp, scan (carried), C-dot
                psy = ps.tile([P, CW], F32, tag="pk", space="PSUM", bufs=1,
                              name="psy")
                for rt in range(NRT):
                    pr = ps.tile([P, CW], F32, tag="pm", space="PSUM", bufs=3,
                                 name="pr")
                    _mm(nc, out=pr[:], lhsT=rrep[:, rt * P:(rt + 1) * P],
                        rhs=delta[:].bitcast(F32R), start=True, stop=True)
                    dA = sbs.tile([P, CW], BF16, tag="dA", bufs=2)
                    nc.scalar.activation(dA[:], pr[:], AF.Exp,
                                         scale=wl["acols"][:, rt:rt + 1])
                    px2 = ps.tile([P, CW], F32, tag="pm", space="PSUM", bufs=3,
                                  name="px2")
                    _mm(nc, out=px2[:], lhsT=rrep[:, rt * P:(rt + 1) * P],
                        rhs=dx[:].bitcast(F32R), start=True, stop=True)
                    dBx = sbs.tile([P, CW], BF16, tag="dBx", bufs=2)
                    nc.vector.tensor_tensor(out=dBx[:], in0=px2[:],
                                            in1=brep[:], op=OP.mult)
                    h_c = sbs.tile([P, CW], BF16, tag="h", bufs=2, name="h_c")
                    init = 0.0 if o == 0 else carry[:, rt:rt + 1]
                    nc.vector.tensor_tensor_scan(
                        h_c[:], dA[:], dBx[:], init, OP.mult, OP.add)
                    if o == 0:
                        nc.gpsimd.tensor_copy(carry[:, rt:rt + 1], h_c[:, CW - 1:CW])
                    hc = sbs.tile([P, CW], F32, tag="hc", bufs=2, name="hc")
                    nc.gpsimd.tensor_tensor(out=hc[:].bitcast(F32R), in0=h_c[:],
                                            in1=crep[:], op=OP.mult)
                    _mm(nc, out=psy[:], lhsT=srep[:, rt * P:(rt + 1) * P],
                        rhs=hc[:], start=(rt == 0), stop=(rt == NRT - 1))

                # y = ysum + Dp*xs; gate with silu(z) = z*rz
                yg = sbc.tile([P, CW], F32, tag="yg")
                nc.vector.scalar_tensor_tensor(
                    out=yg[:].bitcast(F32R), in0=xs_c[:],
                    scalar=wl["dp"][:, :1], in1=psy[:],
                    op0=OP.mult, op1=OP.add)
                nc.gpsimd.tensor_tensor(out=yg[:].bitcast(F32R), in0=yg[:],
                                        in1=z_c[:], op=OP.mult)
                nc.gpsimd.tensor_tensor(out=yg[:].bitcast(F32R), in0=yg[:],
                                        in1=rz_c[:], op=OP.mult)

                # out_proj partial (bf16 payload) + chunked AllReduce
                op_l = drp.tile([DIM, CW], BF16, tag="op_l", bufs=2)
                for d in range(ND):
                    po = ps.tile([P, CW], F32, tag="pk", space="PSUM", bufs=1,
                                 name="po")
                    _mm(nc, out=po[:], lhsT=wl["outw"][:, d * P:(d + 1) * P],
                        rhs=yg[:], start=True, stop=True)
                    oev = sb2.tile([P, CW], BF16, tag="oev")
                    nc.scalar.copy(oev[:], po[:])
                    nc.scalar.dma_start(op_l[d * P:(d + 1) * P, :], oev[:])
                op_r = drp.tile([DIM, CW], BF16, tag="op_r", bufs=2,
                                addr_space="Shared")
                nc.gpsimd.collective_compute(
                    "AllReduce", OP.add, replica_groups=RG,
                    ins=[op_l.opt()], outs=[op_r.opt()])
                return dict(sl=sl, op_r=op_r)

            def do_residual(pend):
                if pend is None:
                    return
                sl, op_r = pend["sl"], pend["op_r"]
                for d in range(ND):
                    art = sb2.tile([P, CW], BF16, tag="art", bufs=4, name="art")
                    nc.sync.dma_start(art[:], op_r[d * P:(d + 1) * P, :])
                    nc.gpsimd.tensor_tensor(out=hT[d][:, sl].bitcast(F32R),
                                            in0=hT[d][:, sl], in1=art[:],
                                            op=OP.add)

            pends = []
            for l in range(N_LAYERS):
                st = front(l, 0)
                for k in range(NT):
                    nxt = front(l, k + 1) if k + 1 < NT else None
                    pends.append(back(l, k, st, None))
                    if len(pends) >= 3:
                        do_residual(pends.pop(0))
                    st = nxt

            # ---- final layernorm + bf16 head, per chunk ----
            # (remaining deferred residuals drain one per LN chunk, just
            # ahead of the chunk that needs them)
            for k in range(NT):
                if k >= 1 and pends:
                    do_residual(pends.pop(0))
                sl = slice(k * CW, (k + 1) * CW)
                pmu = ps.tile([1, CW], F32, tag="pf", space="PSUM", bufs=2)
                for d in range(ND):
                    _mm(nc, out=pmu[:], lhsT=ones_col[:], rhs=hT[d][:, sl],
                        start=(d == 0), stop=(d == ND - 1))
                mu_c = sbc.tile([1, CW], F32, tag="mu", bufs=1)
                nc.scalar.activation(mu_c[:], pmu[:], AF.Copy, scale=1.0 / DIM)
                psq = ps.tile([1, CW], F32, tag="pf", space="PSUM", bufs=2)
                for d in range(ND):
                    hsq = sb2.tile([P, CW], F32, tag="hsq")
                    nc.scalar.activation(hsq[:].bitcast(F32R), hT[d][:, sl],
                                         AF.Square)
                    _mm(nc, out=psq[:], lhsT=ones_col[:], rhs=hsq[:],
                        start=(d == 0), stop=(d == ND - 1))
                ex2_c = sbc.tile([1, CW], F32, tag="ex2", bufs=1)
                nc.scalar.activation(ex2_c[:], psq[:], AF.Copy, scale=1.0 / DIM)
                msq_p = ps.tile([1, CW], F32, tag="pf", space="PSUM", bufs=2)
                nc.scalar.activation(msq_p[:], mu_c[:], AF.Square)
                var_c = ex2_c  # overwrite in place
                nc.vector.tensor_tensor(out=var_c[:], in0=ex2_c[:], in1=msq_p[:],
                                        op=OP.subtract)
                lnv_p = ps.tile([1, CW], F32, tag="pf", space="PSUM", bufs=2)
                nc.scalar.activation(lnv_p[:], var_c[:], AF.Ln, bias=EPS)
                linv_c = sbc.tile([1, CW], F32, tag="linv", bufs=1)
                nc.scalar.activation(linv_c[:].bitcast(F32R), lnv_p[:],
                                     AF.Exp, scale=-0.5)
                minv_c = sbc.tile([1, CW], F32, tag="minv", bufs=1)
                nc.vector.tensor_tensor(out=minv_c[:].bitcast(F32R), in0=mu_c[:],
                                        in1=linv_c[:], op=OP.mult)
                pbi = ps.tile([P, CW], F32, tag="pf", space="PSUM", bufs=2)
                _mm(nc, out=pbi[:], lhsT=ones_row[:], rhs=linv_c[:],
                    start=True, stop=True)
                pbm = ps.tile([P, CW], F32, tag="pf", space="PSUM", bufs=2)
                _mm(nc, out=pbm[:], lhsT=ones_row[:], rhs=minv_c[:],
                    start=True, stop=True)
                hn_c = []
                for d in range(ND):
                    tmp = sb2.tile([P, CW], F32, tag="w512", name="hntmp")
                    nc.vector.tensor_tensor(out=tmp[:], in0=hT[d][:, sl],
                                            in1=pbi[:], op=OP.mult)
                    hnd = sbc.tile([P, CW], BF16, tag=f"hn{d}", name=f"hn{d}")
                    nc.vector.tensor_tensor(out=hnd[:], in0=tmp[:],
                                            in1=pbm[:], op=OP.subtract)
                    hn_c.append(hnd)

                # head: logits[t, v] = hn^T @ head_w^T (bf16), vocab-sharded
                for tt in range(CW // P):
                    trow = k * CW + tt * P
                    for vc in range(NVC):
                        ph = ps.tile([P, VC], F32, tag="ph", space="PSUM", bufs=2)
                        for d in range(ND):
                            nc.tensor.matmul(
                                out=ph[:], lhsT=hn_c[d][:, tt * P:(tt + 1) * P],
                                rhs=hw[d][:, vc * VC:(vc + 1) * VC],
                                start=(d == 0), stop=(d == ND - 1))
                        osb = sb2.tile([P, VC], F32, tag="oev", name="osb")
                        if vc % 2 == 0:
                            nc.vector.tensor_copy(osb[:], ph[:])
                        else:
                            nc.scalar.copy(osb[:], ph[:])
                        nc.sync.dma_start(
                            logits[trow:trow + P, vc * VC:(vc + 1) * VC], osb[:])

    nc.compile()
    return nc


def prep_inputs(inputs):
    """Build the 8 per-core input maps from the full model inputs."""
    x = np.asarray(inputs["x"]).reshape(-1).astype(np.int32)[:, None]  # [T,1]
    embed = np.ascontiguousarray(np.asarray(inputs["embed"], np.float32))
    rms_w = np.asarray(inputs["rms_w"], np.float32)
    in_w = np.asarray(inputs["in_w"], np.float32)
    conv_w = np.asarray(inputs["conv_w"], np.float32)
    conv_b = np.asarray(inputs["conv_b"], np.float32)
    xproj_w = np.asarray(inputs["xproj_w"], np.float32)
    dt_w = np.asarray(inputs["dt_w"], np.float32)
    dt_b = np.asarray(inputs["dt_b"], np.float32)
    A_log = np.asarray(inputs["A_log"], np.float32)
    Dp = np.asarray(inputs["Dp"], np.float32)
    out_w = np.asarray(inputs["out_w"], np.float32)
    ln_g = np.asarray(inputs["ln_g"], np.float32)
    ln_b = np.asarray(inputs["ln_b"], np.float32)
    head_w = np.asarray(inputs["head_w"], np.float32)
    head_b = np.asarray(inputs["head_b"], np.float32)

    rrep = np.zeros((P, NRT * P), np.float32)   # [k=src ch, rt*128 + (c,s)]
    srep = np.zeros((P, NRT * P), np.float32)   # [k=(c,s), rt*128 + out ch]
    for rt in range(NRT):
        for p_ in range(P):
            c, s = divmod(p_, 16)
            rrep[8 * rt + c, rt * P + p_] = 1.0
            srep[p_, rt * P + 8 * rt + c] = 1.0
    rbc = np.zeros((64, 2 * P), np.float32)     # rows 32:64 = dbc B/C window
    for p_ in range(P):
        s = p_ % 16
        rbc[32 + s, p_] = 1.0          # B pattern
        rbc[32 + 16 + s, P + p_] = 1.0  # C pattern

    # fold ln gamma into head_w; ln beta into the host-side bias
    head_w_eff = (head_w * ln_g[None, :]).astype(np.float32)
    head_b_eff = (head_b + head_w.astype(np.float64) @ ln_b.astype(np.float64)
                  ).astype(np.float32)

    in_maps = []
    for c in range(N_CORES):
        cs = slice(c * CH, (c + 1) * CH)
        vs = slice(c * VSH, (c + 1) * VSH)
        m = {
            "tok": x, "embed": embed,
            "rrep": rrep, "rbc": rbc, "srep": srep,
            "ones_row": np.ones((P, P), np.float32),
            "ones_col": np.ones((P, 1), np.float32),
            "headw": np.ascontiguousarray(
                head_w_eff[vs, :].T).astype(bfloat16),
        }
        for l in range(N_LAYERS):
            w_eff = in_w[l] * rms_w[l][None, :]
            A = -np.exp(A_log[l])  # (D_INNER, D_STATE)
            m.update({
                f"inwx{l}": np.ascontiguousarray(w_eff[cs, :].T),
                f"inwz{l}": np.ascontiguousarray(
                    w_eff[D_INNER + c * CH:D_INNER + (c + 1) * CH, :].T),
                f"convw{l}": np.ascontiguousarray(conv_w[l][cs, 0, :]),
                f"convb{l}": np.ascontiguousarray(conv_b[l][cs][:, None]),
                f"xpw{l}": np.ascontiguousarray(xproj_w[l].T[cs, :]),
                f"dtw{l}": np.ascontiguousarray(dt_w[l][cs, :].T),
                f"dtb{l}": np.ascontiguousarray(dt_b[l][cs][:, None]),
                f"acols{l}": np.ascontiguousarray(
                    A[cs, :].reshape(NRT, 8, 16).reshape(NRT, P).T),
                f"dp{l}": np.ascontiguousarray(Dp[l][cs][:, None]),
                f"outw{l}": np.ascontiguousarray(out_w[l][:, cs].T),
            })
        in_maps.append(m)
    return in_maps, head_b_eff


_NC_CACHE = {}


def kernel(**inputs) -> np.ndarray:
    from concourse.bass_utils import run_bass_kernel_spmd
    if "nc" not in _NC_CACHE:
        _NC_CACHE["nc"] = build_program()
    nc = _NC_CACHE["nc"]
    in_maps, head_b_eff = prep_inputs(inputs)
    res = run_bass_kernel_spmd(nc, in_maps, list(range(N_CORES)))
    shards = [res.results[c]["logits"].reshape(B, L, VSH) for c in range(N_CORES)]
    out = np.concatenate(shards, axis=2)
    out += head_b_eff[None, None, :]
    return out.astype(np.float32)


if __name__ == "__main__":
    nc = build_program()
    print("program built ok")


# revision 34
# speedup vs baseline: 1.0505x; 1.0505x over previous
"""Trainium2 Bass kernel for a 2-layer Mamba LM (B=2, L=1024, D=512,
d_inner=1024, d_state=16, vocab=32000) on 8 NeuronCores.

Sharding: d_inner tensor-parallel for the Mamba blocks (each core owns 128
of the 1024 inner channels; AllReduce for x_proj (64x2048) and out_proj
(512x2048) partial sums), vocab-sharded LM head (4000 rows per core, no
collective), embedding gathered per-core via indirect DMA.

v2: the whole network is software-pipelined over 4 token chunks of 512.
Each layer processes chunk k end-to-end (rms -> in_proj -> conv -> x_proj
-> AllReduce -> scan (state carried across chunks) -> gate -> out_proj ->
AllReduce -> residual), so the per-chunk AllReduces overlap compute of the
neighboring chunks.  All big matmuls run fp32r (1 cyc/row).  The LM head
runs in bf16 with fully preloaded weights and consumes the final-layernorm
output chunk by chunk, overlapping the tail of layer 1.
"""
import numpy as np
from ml_dtypes import bfloat16

import concourse.bass as bass
import concourse.bacc as bacc
import concourse.mybir as mybir
import concourse.tile as tile
from concourse.masks import make_identity

# model dims
B, L = 2, 1024
DIM = 512
D_STATE = 16
D_INNER = 1024
DT_RANK = 32
VOCAB = 32000
N_LAYERS = 2
EPS = 1e-5

N_CORES = 8
CH = D_INNER // N_CORES          # 128 local channels
VSH = VOCAB // N_CORES           # 4000 local vocab rows
T = B * L                        # 2048 token columns (batch-major)
P = 128
CW = 512                         # token chunk width
NT = T // CW                     # 4 token chunks
ND = DIM // P                    # 4 dim tiles
NRT = CH // 8                    # 16 row-tiles (8 ch x 16 states each)
VC = 500                         # head vocab chunk
NVC = VSH // VC                  # 8 vocab chunks
F32 = mybir.dt.float32
F32R = mybir.dt.float32r
BF16 = mybir.dt.bfloat16
I32 = mybir.dt.int32
AF = mybir.ActivationFunctionType
OP = mybir.AluOpType
RG = [list(range(N_CORES))]


def _mm(nc, out, lhsT, rhs, **kw):
    if lhsT.dtype == F32:
        lhsT = lhsT.bitcast(F32R)
    if rhs.dtype == F32:
        rhs = rhs.bitcast(F32R)
    nc.tensor.matmul(out=out, lhsT=lhsT, rhs=rhs, **kw)


def build_program():
    nc = bacc.Bacc("TRN2", num_devices=N_CORES)
    # register EPS as a const AP so activation(bias=EPS) works
    _ct = nc.alloc_sbuf_tensor(f"const-float32-{EPS}", [128, 1], F32)
    nc.gpsimd.memset(_ct.ap(), EPS)
    nc.const_aps.aps[(F32, EPS)] = _ct.ap()
    nc.all_engine_barrier()

    # ---- DRAM I/O ----
    tok = nc.dram_tensor("tok", [T, 1], I32, kind="ExternalInput").ap()
    embed = nc.dram_tensor("embed", [VOCAB, DIM], F32, kind="ExternalInput").ap()
    lw = []  # per-layer weights
    for l in range(N_LAYERS):
        lw.append({
            "inwx": nc.dram_tensor(f"inwx{l}", [DIM, CH], F32R, kind="ExternalInput").ap(),
            "inwz": nc.dram_tensor(f"inwz{l}", [DIM, CH], F32R, kind="ExternalInput").ap(),
            "convw": nc.dram_tensor(f"convw{l}", [CH, 4], F32, kind="ExternalInput").ap(),
            "convb": nc.dram_tensor(f"convb{l}", [CH, 1], F32, kind="ExternalInput").ap(),
            "xpw": nc.dram_tensor(f"xpw{l}", [CH, 64], F32R, kind="ExternalInput").ap(),
            "dtw": nc.dram_tensor(f"dtw{l}", [DT_RANK, CH], F32R, kind="ExternalInput").ap(),
            "dtb": nc.dram_tensor(f"dtb{l}", [CH, 1], F32, kind="ExternalInput").ap(),
            "acols": nc.dram_tensor(f"acols{l}", [P, NRT], F32, kind="ExternalInput").ap(),
            "dp": nc.dram_tensor(f"dp{l}", [CH, 1], F32, kind="ExternalInput").ap(),
            "outw": nc.dram_tensor(f"outw{l}", [CH, DIM], F32R, kind="ExternalInput").ap(),
        })
    rrepd = nc.dram_tensor("rrep", [P, NRT * P], F32R, kind="ExternalInput").ap()
    rbcd = nc.dram_tensor("rbc", [64, 2 * P], F32R, kind="ExternalInput").ap()
    srepd = nc.dram_tensor("srep", [P, NRT * P], F32R, kind="ExternalInput").ap()
    onesrd = nc.dram_tensor("ones_row", [P, P], F32R, kind="ExternalInput").ap()
    onescd = nc.dram_tensor("ones_col", [P, 1], F32R, kind="ExternalInput").ap()
    headw = nc.dram_tensor("headw", [DIM, VSH], BF16, kind="ExternalInput").ap()
    logits = nc.dram_tensor("logits", [T, VSH], F32, kind="ExternalOutput").ap()

    with tile.TileContext(nc) as tc:
        with (
            nc.allow_low_precision(reason="fp32r/bf16 rounding intentional; "
                                   "rel-err budget is 2e-2"),
            tc.tile_pool(name="sb1", bufs=1) as sb1,
            tc.tile_pool(name="sb2", bufs=2) as sb2,
            tc.tile_pool(name="sbc", bufs=2) as sbc,
            tc.tile_pool(name="sbs", bufs=2) as sbs,
            tc.tile_pool(name="ps", bufs=3, space="PSUM") as ps,
            tc.tile_pool(name="dram", bufs=2, space="DRAM") as drp,
        ):
            # ---- warmup AllReduce (absorbs CC ring setup latency) ----
            wz = sb1.tile([64, 16], F32)
            nc.vector.memset(wz[:], 0.0)
            warm_l = drp.tile([64, 16], F32, tag="warm_l")
            warm_r = drp.tile([64, 16], F32, tag="warm_r", addr_space="Shared")
            nc.sync.dma_start(warm_l[:], wz[:])
            nc.gpsimd.collective_compute(
                "AllReduce", OP.add, replica_groups=RG,
                ins=[warm_l.opt()], outs=[warm_r.opt()])

            # ---- consts needed by the gather/transpose path ----
            ident = sb1.tile([P, P], F32)
            make_identity(nc, ident[:])
            rrep = sb1.tile([P, NRT * P], F32R)
            nc.sync.dma_start(rrep[:], rrepd[:])
            rbc = sb1.tile([64, 2 * P], F32R)
            nc.sync.dma_start(rbc[:], rbcd[:])
            srep = sb1.tile([P, NRT * P], F32R)
            nc.sync.dma_start(srep[:], srepd[:])
            ones128 = sb1.tile([P, P], F32R)
            nc.sync.dma_start(ones128[:], onesrd[:])
            ones_row = ones128[0:1, :]
            ones_col = sb1.tile([P, 1], F32R)
            nc.sync.dma_start(ones_col[:], onescd[:])

            # ---- embedding gather + transpose into hT (emitted first: it
            # gates the whole layer pipeline) ----
            hT = [sb1.tile([P, T], F32, tag=f"hT{d}", name=f"hT{d}") for d in range(ND)]
            for tt in range(T // P):
                idx_t = sb2.tile([P, 1], I32, tag="idx")
                nc.sync.dma_start(idx_t[:], tok[tt * P:(tt + 1) * P, :])
                g_t = sb2.tile([P, DIM], F32, tag="w512")
                nc.gpsimd.indirect_dma_start(
                    out=g_t[:], out_offset=None, in_=embed[:],
                    in_offset=bass.IndirectOffsetOnAxis(ap=idx_t[:, :1], axis=0),
                )
                for d in range(ND):
                    gT_ps = ps.tile([P, P], F32, tag="pf", space="PSUM", bufs=2)
                    nc.tensor.transpose(out=gT_ps[:], in_=g_t[:, d * P:(d + 1) * P],
                                        identity=ident[:])
                    dst = hT[d][:, tt * P:(tt + 1) * P].bitcast(F32R)
                    if (tt + d) % 2 == 0:
                        nc.vector.tensor_copy(dst, gT_ps[:])
                    else:
                        nc.scalar.copy(dst, gT_ps[:])

            # ---- layer weights (sync queue), head weights (scalar queue) ----
            w = []
            for l in range(N_LAYERS):
                d = {}
                for k, ap in lw[l].items():
                    if k in ("inwx", "inwz"):
                        tl_ = []
                        for kk in range(ND):
                            t_ = sb1.tile([P, CH], F32R, tag=f"{k}{l}_{kk}")
                            nc.sync.dma_start(t_[:], ap[kk * P:(kk + 1) * P, :])
                            tl_.append(t_)
                        d[k] = tl_
                    else:
                        t_ = sb1.tile(list(ap.shape), ap.dtype, tag=f"{k}{l}")
                        nc.sync.dma_start(t_[:], ap[:])
                        d[k] = t_
                # negated conv bias for the sigmoid-via-exp path
                cbn = sb1.tile([CH, 1], F32, tag=f"convbn{l}", name="cbn")
                nc.vector.tensor_scalar_mul(cbn[:], d["convb"][:], -1.0)
                d["convbn"] = cbn
                w.append(d)
            # head weights, fully preloaded (bf16, 4x[128,4000])
            hw = []
            for d in range(ND):
                hw_t = sb1.tile([P, VSH], BF16, tag=f"hw{d}", name=f"hw{d}")
                nc.scalar.dma_start(hw_t[:], headw[d * P:(d + 1) * P, :])
                hw.append(hw_t)

            # conv inputs, per batch (3-col causal halo at the front)
            xc = [sb1.tile([P, L + 3], F32, tag=f"xc{b_}", name=f"xc{b_}")
                  for b_ in range(B)]
            # scan carry columns (one per row-tile), written at even chunks
            carry = sbc.tile([P, NRT], F32, tag="carry", name="carry")

            # ---- layers: two-stage software pipeline over 4 chunks ----
            # front(l,k): stats -> in_proj -> conv -> x_proj -> AR_dbc
            # back(l,k):  delta -> scan -> gate -> out_proj -> AR_op
            # Emission order front(l,k+1) before back(l,k) puts AR_dbc(k+1)
            # ahead of AR_op(k) on the CC stream so the scan of k+1 never
            # waits on the out_proj reduce of k.  Residual adds are deferred
            # one slot so their AR wait never blocks an engine queue.

            def front(l, k):
                wl = w[l]
                sl = slice(k * CW, (k + 1) * CW)
                b_, o = divmod(k * CW, L)
                ssp = ps.tile([1, CW], F32, tag="pf", space="PSUM", bufs=2,
                              name="ssp")
                for d in range(ND):
                    hsq = sb2.tile([P, CW], F32, tag="hsq")
                    nc.scalar.activation(hsq[:].bitcast(F32R), hT[d][:, sl],
                                         AF.Square)
                    _mm(nc, out=ssp[:], lhsT=ones_col[:], rhs=hsq[:],
                        start=(d == 0), stop=(d == ND - 1))
                # inv = (mean+eps)^-1/2 = exp(-0.5*ln(.)) - exp/ln table only
                lnm_c = sbc.tile([1, CW], F32, tag="lnm", bufs=1)
                nc.scalar.activation(lnm_c[:], ssp[:],
                                     AF.Ln, bias=EPS, scale=1.0 / DIM)
                inv_c = sbc.tile([1, CW], F32, tag="inv")
                nc.scalar.activation(inv_c[:].bitcast(F32R), lnm_c[:],
                                     AF.Exp, scale=-0.5)
                bp = ps.tile([P, CW], F32, tag="pf", space="PSUM", bufs=2,
                             name="bp")
                _mm(nc, out=bp[:], lhsT=ones_row[:], rhs=inv_c[:],
                    start=True, stop=True)
                bcinv = sbc.tile([P, CW], F32, tag="bcinv")
                nc.scalar.copy(bcinv[:], bp[:])

                # in_proj (rms gamma folded into W; scale by inv on evac)
                if o == 0:
                    nc.vector.memset(xc[b_][:, 0:3], 0.0)
                px = ps.tile([P, CW], F32, tag="pf", space="PSUM", bufs=2,
                             name="px")
                pz = ps.tile([P, CW], F32, tag="pf", space="PSUM", bufs=2,
                             name="pz")
                for d in range(ND):
                    _mm(nc, out=px[:], lhsT=wl["inwx"][d][:],
                        rhs=hT[d][:, sl], start=(d == 0), stop=(d == ND - 1))
                for d in range(ND):
                    _mm(nc, out=pz[:], lhsT=wl["inwz"][d][:],
                        rhs=hT[d][:, sl], start=(d == 0), stop=(d == ND - 1))
                nc.vector.tensor_tensor(out=xc[b_][:, 3 + o:3 + o + CW],
                                        in0=px[:], in1=bcinv[:], op=OP.mult)
                z_c = sbc.tile([P, CW], F32, tag="z")
                nc.vector.tensor_tensor(out=z_c[:], in0=pz[:],
                                        in1=bcinv[:], op=OP.mult)
                # silu(z) denominator: 1/(1+exp(-z)) via exp table + DVE
                ez = sbc.tile([P, CW], F32, tag="ez", bufs=1)
                nc.scalar.activation(ez[:], z_c[:], AF.Exp, scale=-1.0)
                nc.scalar.activation(ez[:], ez[:], AF.Copy, bias=1.0)
                rz_c = sbc.tile([P, CW], F32, tag="rz")
                nc.vector.reciprocal(rz_c[:], ez[:])

                # causal depthwise conv + silu -> xs_c
                cv = sbc.tile([P, CW], F32, tag="cv", bufs=1, name="cv")
                nc.vector.tensor_scalar_mul(cv[:], xc[b_][:, o:o + CW],
                                            wl["convw"][:, 0:1])
                for kk in (1, 2, 3):
                    nc.vector.scalar_tensor_tensor(
                        out=cv[:], in0=xc[b_][:, o + kk:o + kk + CW],
                        scalar=wl["convw"][:, kk:kk + 1], in1=cv[:],
                        op0=OP.mult, op1=OP.add)
                ec = sb2.tile([P, CW], F32, tag="w512", name="ec")
                nc.scalar.activation(ec[:], cv[:], AF.Exp, scale=-1.0,
                                     bias=wl["convbn"][:, :1])
                nc.scalar.activation(ec[:], ec[:], AF.Copy, bias=1.0)
                rc = sb2.tile([P, CW], F32, tag="w512", name="rc")
                nc.vector.reciprocal(rc[:], ec[:])
                xs_c = sbc.tile([P, CW], F32, tag="xs")
                nc.vector.scalar_tensor_tensor(
                    out=xs_c[:].bitcast(F32R), in0=cv[:],
                    scalar=wl["convb"][:, :1], in1=rc[:],
                    op0=OP.add, op1=OP.mult)

                # x_proj partial + chunked AllReduce (trigger via scalar DMA)
                pd_ = ps.tile([64, CW], F32, tag="pf", space="PSUM", bufs=2,
                              name="pd_")
                _mm(nc, out=pd_[:], lhsT=wl["xpw"][:], rhs=xs_c[:],
                    start=True, stop=True)
                dbev = sb2.tile([64, CW], F32, tag="w512", name="dbev")
                nc.scalar.copy(dbev[:], pd_[:])
                dbc_l = drp.tile([64, CW], F32, tag="dbc_l", bufs=2)
                nc.scalar.dma_start(dbc_l[:], dbev[:])
                dbc_r = drp.tile([64, CW], F32, tag="dbc_r", bufs=2,
                                 addr_space="Shared")
                nc.gpsimd.collective_compute(
                    "AllReduce", OP.add, replica_groups=RG,
                    ins=[dbc_l.opt()], outs=[dbc_r.opt()])
                return dict(sl=sl, b_=b_, o=o, xs=xs_c, z=z_c, rz=rz_c,
                            dbc_r=dbc_r)

            def back(l, k, st, _unused):
                wl = w[l]
                sl, o = st["sl"], st["o"]
                xs_c, z_c, rz_c = st["xs"], st["z"], st["rz"]
                dbc = sbc.tile([64, CW], F32R, tag="dbc")
                nc.sync.dma_start(dbc[:], st["dbc_r"][:].bitcast(F32R))

                # delta = softplus(dt_w @ dbc[:32] + dt_b)
                pt = ps.tile([P, CW], F32, tag="pm", space="PSUM", bufs=3,
                             name="pt")
                _mm(nc, out=pt[:], lhsT=wl["dtw"][:], rhs=dbc[0:32, :],
                    start=True, stop=True)
                et = sb2.tile([P, CW], F32, tag="w512", name="et")
                nc.scalar.activation(et[:], pt[:], AF.Exp,
                                     bias=wl["dtb"][:, :1])
                delta = sbc.tile([P, CW], F32, tag="delta")
                nc.scalar.activation(delta[:].bitcast(F32R), et[:], AF.Ln,
                                     bias=1.0)
                dx = sbc.tile([P, CW], F32, tag="dx")
                nc.gpsimd.tensor_tensor(out=dx[:].bitcast(F32R), in0=delta[:],
                                        in1=xs_c[:], op=OP.mult)

                # B/C broadcast into (c,s)-row layout
                pb = ps.tile([P, CW], F32, tag="pm", space="PSUM", bufs=3,
                             name="pb")
                _mm(nc, out=pb[:], lhsT=rbc[32:64, 0:P], rhs=dbc[32:64, :],
                    start=True, stop=True)
                brep = sbc.tile([P, CW], F32, tag="brep")
                nc.scalar.copy(brep[:], pb[:])
                pc = ps.tile([P, CW], F32, tag="pm", space="PSUM", bufs=3,
                             name="pc")
                _mm(nc, out=pc[:], lhsT=rbc[32:64, P:2 * P], rhs=dbc[32:64, :],
                    start=True, stop=True)
                crep = sbc.tile([P, CW], F32, tag="crep")
                nc.scalar.copy(crep[:], pc[:])

                # row-tiles: replicate delta/dx, exp, scan (carried), C-dot
                psy = ps.tile([P, CW], F32, tag="pk", space="PSUM", bufs=1,
                              name="psy")
                for rt in range(NRT):
                    pr = ps.tile([P, CW], F32, tag="pm", space="PSUM", bufs=3,
                                 name="pr")
                    _mm(nc, out=pr[:], lhsT=rrep[:, rt * P:(rt + 1) * P],
                        rhs=delta[:].bitcast(F32R), start=True, stop=True)
                    dA = sbs.tile([P, CW], BF16, tag="dA", bufs=2)
                    nc.scalar.activation(dA[:], pr[:], AF.Exp,
                                         scale=wl["acols"][:, rt:rt + 1])
                    px2 = ps.tile([P, CW], F32, tag="pm", space="PSUM", bufs=3,
                                  name="px2")
                    _mm(nc, out=px2[:], lhsT=rrep[:, rt * P:(rt + 1) * P],
                        rhs=dx[:].bitcast(F32R), start=True, stop=True)
                    dBx = sbs.tile([P, CW], BF16, tag="dBx", bufs=2)
                    nc.vector.tensor_tensor(out=dBx[:], in0=px2[:],
                                            in1=brep[:], op=OP.mult)
                    h_c = sbs.tile([P, CW], BF16, tag="h", bufs=2, name="h_c")
                    init = 0.0 if o == 0 else carry[:, rt:rt + 1]
                    nc.vector.tensor_tensor_scan(
                        h_c[:], dA[:], dBx[:], init, OP.mult, OP.add)
                    if o == 0:
                        nc.gpsimd.tensor_copy(carry[:, rt:rt + 1], h_c[:, CW - 1:CW])
                    hc = sbs.tile([P, CW], F32, tag="hc", bufs=2, name="hc")
                    nc.gpsimd.tensor_tensor(out=hc[:].bitcast(F32R), in0=h_c[:],
                                            in1=crep[:], op=OP.mult)
                    _mm(nc, out=psy[:], lhsT=srep[:, rt * P:(rt + 1) * P],
                        rhs=hc[:], start=(rt == 0), stop=(rt == NRT - 1))

                # y = ysum + Dp*xs; gate with silu(z) = z*rz
                yg = sbc.tile([P, CW], F32, tag="yg")
                nc.vector.scalar_tensor_tensor(
                    out=yg[:].bitcast(F32R), in0=xs_c[:],
                    scalar=wl["dp"][:, :1], in1=psy[:],
                    op0=OP.mult, op1=OP.add)
                nc.gpsimd.tensor_tensor(out=yg[:].bitcast(F32R), in0=yg[:],
                                        in1=z_c[:], op=OP.mult)
                nc.gpsimd.tensor_tensor(out=yg[:].bitcast(F32R), in0=yg[:],
                                        in1=rz_c[:], op=OP.mult)

                # out_proj partial (bf16 payload) + chunked AllReduce
                op_l = drp.tile([DIM, CW], BF16, tag="op_l", bufs=2)
                for d in range(ND):
                    po = ps.tile([P, CW], F32, tag="pk", space="PSUM", bufs=1,
                                 name="po")
                    _mm(nc, out=po[:], lhsT=wl["outw"][:, d * P:(d + 1) * P],
                        rhs=yg[:], start=True, stop=True)
                    oev = sb2.tile([P, CW], BF16, tag="oev")
                    nc.scalar.copy(oev[:], po[:])
                    nc.scalar.dma_start(op_l[d * P:(d + 1) * P, :], oev[:])
                op_r = drp.tile([DIM, CW], BF16, tag="op_r", bufs=2,
                                addr_space="Shared")
                nc.gpsimd.collective_compute(
                    "AllReduce", OP.add, replica_groups=RG,
                    ins=[op_l.opt()], outs=[op_r.opt()])
                return dict(sl=sl, op_r=op_r)

            def do_residual(pend):
                if pend is None:
                    return
                sl, op_r = pend["sl"], pend["op_r"]
                # deprioritize so the scheduler places these after the
                # current chunk's scan work (their AllReduce lands late)
                with tc.high_priority(offset=-400):
                    for d in range(ND):
                        art = sb2.tile([P, CW], BF16, tag="art", bufs=4,
                                       name="art")
                        nc.sync.dma_start(art[:], op_r[d * P:(d + 1) * P, :])
                        nc.vector.tensor_tensor(
                            out=hT[d][:, sl].bitcast(F32R),
                            in0=hT[d][:, sl], in1=art[:], op=OP.add)

            pends = []
            for l in range(N_LAYERS):
                st = front(l, 0)
                for k in range(NT):
                    nxt = front(l, k + 1) if k + 1 < NT else None
                    pends.append(back(l, k, st, None))
                    if len(pends) >= 3:
                        do_residual(pends.pop(0))
                    st = nxt

            # ---- final layernorm + bf16 head, per chunk ----
            # (remaining deferred residuals drain one per LN chunk, just
            # ahead of the chunk that needs them)
            for k in range(NT):
                if k >= 1 and pends:
                    do_residual(pends.pop(0))
                sl = slice(k * CW, (k + 1) * CW)
                pmu = ps.tile([1, CW], F32, tag="pf", space="PSUM", bufs=2)
                for d in range(ND):
                    _mm(nc, out=pmu[:], lhsT=ones_col[:], rhs=hT[d][:, sl],
                        start=(d == 0), stop=(d == ND - 1))
                mu_c = sbc.tile([1, CW], F32, tag="mu", bufs=1)
                nc.scalar.activation(mu_c[:], pmu[:], AF.Copy, scale=1.0 / DIM)
                psq = ps.tile([1, CW], F32, tag="pf", space="PSUM", bufs=2)
                for d in range(ND):
                    hsq = sb2.tile([P, CW], F32, tag="hsq")
                    nc.scalar.activation(hsq[:].bitcast(F32R), hT[d][:, sl],
                                         AF.Square)
                    _mm(nc, out=psq[:], lhsT=ones_col[:], rhs=hsq[:],
                        start=(d == 0), stop=(d == ND - 1))
                ex2_c = sbc.tile([1, CW], F32, tag="ex2", bufs=1)
                nc.scalar.activation(ex2_c[:], psq[:], AF.Copy, scale=1.0 / DIM)
                msq_p = ps.tile([1, CW], F32, tag="pf", space="PSUM", bufs=2)
                nc.scalar.activation(msq_p[:], mu_c[:], AF.Square)
                var_c = ex2_c  # overwrite in place
                nc.vector.tensor_tensor(out=var_c[:], in0=ex2_c[:], in1=msq_p[:],
                                        op=OP.subtract)
                lnv_p = ps.tile([1, CW], F32, tag="pf", space="PSUM", bufs=2)
                nc.scalar.activation(lnv_p[:], var_c[:], AF.Ln, bias=EPS)
                linv_c = sbc.tile([1, CW], F32, tag="linv", bufs=1)
                nc.scalar.activation(linv_c[:].bitcast(F32R), lnv_p[:],
                                     AF.Exp, scale=-0.5)
                minv_c = sbc.tile([1, CW], F32, tag="minv", bufs=1)
                nc.vector.tensor_tensor(out=minv_c[:].bitcast(F32R), in0=mu_c[:],
                                        in1=linv_c[:], op=OP.mult)
                pbi = ps.tile([P, CW], F32, tag="pf", space="PSUM", bufs=2)
                _mm(nc, out=pbi[:], lhsT=ones_row[:], rhs=linv_c[:],
                    start=True, stop=True)
                pbm = ps.tile([P, CW], F32, tag="pf", space="PSUM", bufs=2)
                _mm(nc, out=pbm[:], lhsT=ones_row[:], rhs=minv_c[:],
                    start=True, stop=True)
                hn_c = []
                for d in range(ND):
                    tmp = sb2.tile([P, CW], F32, tag="w512", name="hntmp")
                    nc.vector.tensor_tensor(out=tmp[:], in0=hT[d][:, sl],
                                            in1=pbi[:], op=OP.mult)
                    hnd = sbc.tile([P, CW], BF16, tag=f"hn{d}", name=f"hn{d}")
                    nc.vector.tensor_tensor(out=hnd[:], in0=tmp[:],
                                            in1=pbm[:], op=OP.subtract)
                    hn_c.append(hnd)

                # head: logits[t, v] = hn^T @ head_w^T (bf16), vocab-sharded
                for tt in range(CW // P):
                    trow = k * CW + tt * P
                    for vc in range(NVC):
                        ph = ps.tile([P, VC], F32, tag="ph", space="PSUM", bufs=2)
                        for d in range(ND):
                            nc.tensor.matmul(
                                out=ph[:], lhsT=hn_c[d][:, tt * P:(tt + 1) * P],
                                rhs=hw[d][:, vc * VC:(vc + 1) * VC],
                                start=(d == 0), stop=(d == ND - 1))
                        osb = sb2.tile([P, VC], F32, tag="oev", name="osb")
                        if vc % 2 == 0:
                            nc.vector.tensor_copy(osb[:], ph[:])
                        else:
                            nc.scalar.copy(osb[:], ph[:])
                        nc.sync.dma_start(
                            logits[trow:trow + P, vc * VC:(vc + 1) * VC], osb[:])

    nc.compile()
    return nc


def prep_inputs(inputs):
    """Build the 8 per-core input maps from the full model inputs."""
    x = np.asarray(inputs["x"]).reshape(-1).astype(np.int32)[:, None]  # [T,1]
    embed = np.ascontiguousarray(np.asarray(inputs["embed"], np.float32))
    rms_w = np.asarray(inputs["rms_w"], np.float32)
    in_w = np.asarray(inputs["in_w"], np.float32)
    conv_w = np.asarray(inputs["conv_w"], np.float32)
    conv_b = np.asarray(inputs["conv_b"], np.float32)
    xproj_w = np.asarray(inputs["xproj_w"], np.float32)
    dt_w = np.asarray(inputs["dt_w"], np.float32)
    dt_b = np.asarray(inputs["dt_b"], np.float32)
    A_log = np.asarray(inputs["A_log"], np.float32)
    Dp = np.asarray(inputs["Dp"], np.float32)
    out_w = np.asarray(inputs["out_w"], np.float32)
    ln_g = np.asarray(inputs["ln_g"], np.float32)
    ln_b = np.asarray(inputs["ln_b"], np.float32)
    head_w = np.asarray(inputs["head_w"], np.float32)
    head_b = np.asarray(inputs["head_b"], np.float32)

    rrep = np.zeros((P, NRT * P), np.float32)   # [k=src ch, rt*128 + (c,s)]
    srep = np.zeros((P, NRT * P), np.float32)   # [k=(c,s), rt*128 + out ch]
    for rt in range(NRT):
        for p_ in range(P):
            c, s = divmod(p_, 16)
            rrep[8 * rt + c, rt * P + p_] = 1.0
            srep[p_, rt * P + 8 * rt + c] = 1.0
    rbc = np.zeros((64, 2 * P), np.float32)     # rows 32:64 = dbc B/C window
    for p_ in range(P):
        s = p_ % 16
        rbc[32 + s, p_] = 1.0          # B pattern
        rbc[32 + 16 + s, P + p_] = 1.0  # C pattern

    # fold ln gamma into head_w; ln beta into the host-side bias
    head_w_eff = (head_w * ln_g[None, :]).astype(np.float32)
    head_b_eff = (head_b + head_w.astype(np.float64) @ ln_b.astype(np.float64)
                  ).astype(np.float32)

    in_maps = []
    for c in range(N_CORES):
        cs = slice(c * CH, (c + 1) * CH)
        vs = slice(c * VSH, (c + 1) * VSH)
        m = {
            "tok": x, "embed": embed,
            "rrep": rrep, "rbc": rbc, "srep": srep,
            "ones_row": np.ones((P, P), np.float32),
            "ones_col": np.ones((P, 1), np.float32),
            "headw": np.ascontiguousarray(
                head_w_eff[vs, :].T).astype(bfloat16),
        }
        for l in range(N_LAYERS):
            w_eff = in_w[l] * rms_w[l][None, :]
            A = -np.exp(A_log[l])  # (D_INNER, D_STATE)
            m.update({
                f"inwx{l}": np.ascontiguousarray(w_eff[cs, :].T),
                f"inwz{l}": np.ascontiguousarray(
                    w_eff[D_INNER + c * CH:D_INNER + (c + 1) * CH, :].T),
                f"convw{l}": np.ascontiguousarray(conv_w[l][cs, 0, :]),
                f"convb{l}": np.ascontiguousarray(conv_b[l][cs][:, None]),
                f"xpw{l}": np.ascontiguousarray(xproj_w[l].T[cs, :]),
                f"dtw{l}": np.ascontiguousarray(dt_w[l][cs, :].T),
                f"dtb{l}": np.ascontiguousarray(dt_b[l][cs][:, None]),
                f"acols{l}": np.ascontiguousarray(
                    A[cs, :].reshape(NRT, 8, 16).reshape(NRT, P).T),
                f"dp{l}": np.ascontiguousarray(Dp[l][cs][:, None]),
                f"outw{l}": np.ascontiguousarray(out_w[l][:, cs].T),
            })
        in_maps.append(m)
    return in_maps, head_b_eff


_NC_CACHE = {}


def kernel(**inputs) -> np.ndarray:
    from concourse.bass_utils import run_bass_kernel_spmd
    if "nc" not in _NC_CACHE:
        _NC_CACHE["nc"] = build_program()
    nc = _NC_CACHE["nc"]
    in_maps, head_b_eff = prep_inputs(inputs)
    res = run_bass_kernel_spmd(nc, in_maps, list(range(N_CORES)))
    shards = [res.results[c]["logits"].reshape(B, L, VSH) for c in range(N_CORES)]
    out = np.concatenate(shards, axis=2)
    out += head_b_eff[None, None, :]
    return out.astype(np.float32)


if __name__ == "__main__":
    nc = build_program()
    print("program built ok")
